# revision 43
# baseline (speedup 1.0000x reference)
# Trainium2 Bass kernel for nn_MultiHeadAttention_29154238005976 (ACAT-style conv-augmented MHA).
#
# Reference computation (B=4, L=1024, D=1024, H=16, DK=64):
#   q/k/v projections; q,k augmented by a "scrambled" depthwise-dense conv
#   (torch-style raw reshapes (b,h,l,dk)->(b, h*dk, l) scramble time/channels);
#   softmax attention per head; output projection.
#
# Sharding: 8 cores = 4 batches x 2 head-halves. All cores run an IDENTICAL
# program; per-core differences are pushed into host-side data permutations:
#   - time permutation sigma(l) = l XOR 8g applied to Q/K/V rows (g = head-half)
#   - channel permutation pi(c) = c XOR 512g applied to WQ/WK columns and to
#     conv kernel rows+columns
#   - conv "time" halves then both map to program range l2' in [0,512)
#   - boundary sig columns get a per-core 0/1 mask (true zero-padding vs
#     wrapped real data)
# Host gathers the 8 partial outputs, un-permutes rows, sums batch pairs, +bfc.
#
# Scramble path: the projection psum tiles are copied (f32->bf16) to SBUF
# staging tiles, staged to DRAM in a gather-friendly layout
# qp2[jh, il, ihi, jl]  (qp row l = 16*il + jh, col c = 64*ihi + jl),
# so each sig gather (dt, hib) is a 3-dim DMA and the wrap block needs no
# DMA at all for flen=2 (only cols 62:64 are ever read; they equal cols
# 1086:1088 masked by em).
import numpy as np
import ml_dtypes

import concourse.bass as bass
import concourse.mybir as mybir
import concourse.tile as tile
from concourse import bacc

bf16 = ml_dtypes.bfloat16
F32 = mybir.dt.float32
BF16 = mybir.dt.bfloat16
AF = mybir.ActivationFunctionType
OP = mybir.AluOpType

B, L, DM, H, DK = 4, 1024, 1024, 16, 64
FMAX = 4
FILTER_LENGTHS = (2, 4)
N_CORES = 8

_CACHE = {}


# ----------------------------------------------------------------------------
# program builder
# ----------------------------------------------------------------------------
def _build(flen: int, zero_bias: bool = False) -> bass.Bass:
    use_tres = (flen == 2)  # residual via PE-transpose folded into conv psum
    nc = bacc.Bacc("TRN2", target_bir_lowering=False, debug=False)

    def din(name, shape, dt):
        return nc.dram_tensor(name, list(shape), dt, kind="ExternalInput").ap()

    Qt_d = din("Qt", (DM, L), BF16)
    Kt_d = din("Kt", (DM, L), BF16)
    Vt_d = din("Vt", (DM, L), BF16)
    Wq_d = din("WQ", (DM, DM), BF16)
    Wk_d = din("WK", (DM, DM), BF16)
    Wv_d = din("WV", (DM, 512), BF16)
    Wfc_d = din("Wfc", (512, DM), BF16)
    cwq_d = din("cwq", (8, 128, flen * 1024), BF16)
    cwk_d = din("cwk", (8, 128, flen * 1024), BF16)
    bQn_d = din("bQn", (1, DM), BF16)
    bKn_d = din("bKn", (1, DM), BF16)
    bVn_d = din("bVn", (1, 512), BF16)
    bQt_d = din("bQt", (128, 4), F32)
    bKt_d = din("bKt", (128, 4), F32)
    em_d = din("emask", (128, 4), F32)
    out_d = nc.dram_tensor("out", [L, DM], BF16, kind="ExternalOutput").ap()

    njh = 16 if use_tres else 9        # gathered 64-col j-bands per (dt, hib)
    SIGW = 64 + 64 * njh               # 1088 (flen=2) / 640 (flen=4)

    with tile.TileContext(nc) as tc:
        sb = tc.alloc_tile_pool(name="sb", bufs=1)
        dr = tc.alloc_tile_pool(name="dr", bufs=1, space="DRAM")
        psA = tc.alloc_tile_pool(name="psA", bufs=1, space="PSUM")

        # ---- tiny setup ------------------------------------------------
        em_sb = sb.tile([128, 4], F32, name="em_sb")
        ones1_sb = sb.tile([1, 128], BF16, name="ones1_sb")
        nc.vector.memset(ones1_sb, 1.0)
        if not zero_bias:
            bQn_sb = sb.tile([1, DM], BF16, name="bQn_sb")
            nc.sync.dma_start(bQn_sb, bQn_d)
            bKn_sb = sb.tile([1, DM], BF16, name="bKn_sb")
            nc.sync.dma_start(bKn_sb, bKn_d)
            bVn_sb = sb.tile([1, 512], BF16, name="bVn_sb")
            nc.sync.dma_start(bVn_sb, bVn_d)
        else:
            bQn_sb = bKn_sb = bVn_sb = None
        if not use_tres:
            bQt_sb = sb.tile([128, 4], F32, name="bQt_sb")
            nc.sync.dma_start(bQt_sb, bQt_d)
            bKt_sb = sb.tile([128, 4], F32, name="bKt_sb")
            nc.sync.dma_start(bKt_sb, bKt_d)

        # ---- big loads: first-use order, split across SP and Pool ------
        # SP/HWDGE carries Qt, Kt, Wq(cb1), Wk, Vt, Wv, Wfc, cw.
        # Pool/SWDGE carries Wq(cb0) so the first matmuls aren't serialized
        # behind the SP issue queue.
        Qt_sb = sb.tile([128, 8, L], BF16, tag="qkvt", bufs=2, name="Qt_sb")
        Wq_sb = sb.tile([128, 8, DM], BF16, name="Wq_sb")
        qtv = Qt_d.rearrange("(a p) c -> p a c", p=128)
        wqv = Wq_d.rearrange("(a p) c -> p a c", p=128)
        nc.sync.dma_start(Qt_sb[:, 0, 0:128], qtv[:, 0, 0:128])
        nc.sync.dma_start(Qt_sb[:, 0, 128:1024], qtv[:, 0, 128:1024])
        nc.sync.dma_start(em_sb, em_d)
        for a in range(8):
            if a % 2 == 0:
                nc.gpsimd.dma_start(Wq_sb[:, a, 0:512], wqv[:, a, 0:512])
            if a > 0:
                nc.sync.dma_start(Qt_sb[:, a], qtv[:, a])
        for a in range(1, 8, 2):
            nc.sync.dma_start(Wq_sb[:, a, 0:512], wqv[:, a, 0:512])
        Wk_sb = sb.tile([128, 8, DM], BF16, name="Wk_sb")
        Kt_sb = sb.tile([128, 8, L], BF16, tag="qkvt", bufs=2, name="Kt_sb")
        wkv = Wk_d.rearrange("(a p) c -> p a c", p=128)
        ktv = Kt_d.rearrange("(a p) c -> p a c", p=128)
        for a in range(8):
            nc.gpsimd.dma_start(Wk_sb[:, a, 0:512], wkv[:, a, 0:512])
            nc.sync.dma_start(Wq_sb[:, a, 512:1024], wqv[:, a, 512:1024])
            nc.sync.dma_start(Kt_sb[:, a], ktv[:, a])
            nc.sync.dma_start(Wk_sb[:, a, 512:1024], wkv[:, a, 512:1024])

        qT_sb = sb.tile([128, 4, L], BF16, name="qT_sb")
        kT_sb = sb.tile([128, 4, L], BF16, name="kT_sb")
        ctxT_sb = sb.tile([128, 4, L], BF16, name="ctxT_sb")
        sigq_sb = sb.tile([128, 8, SIGW], BF16, name="sigq_sb")
        sigk_sb = sb.tile([128, 8, SIGW], BF16, name="sigk_sb")
        if use_tres:
            from concourse.masks import make_identity
            ident_sb = sb.tile([128, 128], BF16, name="ident_sb")
            make_identity(nc, ident_sb)
        vpa_sb = sb.tile([128, 8, 520], BF16, name="vpa_sb")
        vpa_r = vpa_sb.rearrange("p lb (hh c) -> p lb hh c", c=65)
        nc.vector.memset(vpa_r[:, :, :, 64], 1.0)  # the denominator "ones" column

        # staging DRAM in scramble layout [jh, il, ihi, jl]
        qp2_d = dr.tile([16, 64, 16, 64], BF16, name="qp2_d")
        kp2_d = dr.tile([16, 64, 16, 64], BF16, name="kp2_d")

        # ---- phase helpers ---------------------------------------------
        def proj_tile(Xt_sb, W_sb, bn_sb, x2_d, pfx, wq, cb, lb):
            """One l/c tile of x = X @ W (+b), staged to DRAM in scramble layout."""
            x2w = x2_d.rearrange("jh il ihi jl -> il jh (ihi jl)")
            ps = psA.tile([128, 512], F32, tag="mm", bufs=2,
                          name=f"ps_{pfx}_{cb}_{lb}")
            if not zero_bias:
                nc.tensor.matmul(ps, ones1_sb[0:1, :],
                                 bn_sb[0:1, cb * 512:cb * 512 + 512],
                                 start=True, stop=False)
            for dm in range(8):
                nc.tensor.matmul(
                    ps,
                    Xt_sb[:, dm, lb * 128:lb * 128 + 128],
                    W_sb[:, dm, cb * 512:cb * 512 + 512],
                    start=(zero_bias and dm == 0), stop=(dm == 7))
            st = sb.tile([128, 512], BF16, tag="stage", bufs=10,
                         name=f"st_{pfx}_{cb}_{lb}")
            nc.vector.tensor_copy(st, ps)
            # st partition p = 16*il2 + jh iterates in the same linear
            # order as the dst view's (il2, jh) dims
            wq.dma_start(
                x2w[8 * lb:8 * lb + 8, :, 512 * cb:512 * cb + 512], st)

        def proj_nat(Xt_sb, W_sb, bn_sb, x2_d, pfx, wq):
            for cb in range(2):
                for lb in range(8):
                    proj_tile(Xt_sb, W_sb, bn_sb, x2_d, pfx, wq, cb, lb)

        def scramble(x2_d, sig_sb, pfx, pool_share, dts=range(8)):
            """sig[64 hib + il, dt, 64 + 64 jh + jl] <- qp[16 il + jh, 64 ihi + jl]
            (ihi = 2 dt + hib); boundary cols 62:64 fixed up separately."""
            x2g = x2_d.rearrange("jh il ihi jl -> il jh ihi jl")
            for dt in dts:
                for hib in range(2):
                    ihi = 2 * dt + hib
                    dst = sig_sb[64 * hib:64 * hib + 64, dt, 64:64 + 64 * njh]
                    q = nc.gpsimd if (pool_share and dt < 4) else nc.scalar
                    q.dma_start(
                        dst.rearrange("p (jh jl) -> p jh jl", jl=64),
                        x2g[:, 0:njh, ihi])
                    if not use_tres:
                        # flen=4: cols 62:64 (j = 1022:1024) lie outside the
                        # gathered bands; fetch the 2-col wrap strip directly.
                        nc.scalar.dma_start(sig_sb[64 * hib:64 * hib + 64, dt, 62:64],
                                            x2g[:, 15, ihi, 62:64])

        def sig_fixup(sig_sb):
            if use_tres:
                # wrap cols 62:64 duplicate cols 1086:1088; apply pad mask em
                nc.gpsimd.tensor_tensor(
                    sig_sb[:, :, 62:64], sig_sb[:, :, 1086:1088],
                    em_sb[:, None, 0:2].to_broadcast((128, 8, 2)), OP.mult)
            else:
                nc.gpsimd.tensor_tensor(
                    sig_sb[:, :, 62:64], sig_sb[:, :, 62:64],
                    em_sb[:, None, 0:2].to_broadcast((128, 8, 2)), OP.mult)
                nc.gpsimd.tensor_tensor(
                    sig_sb[:, :, 576:578], sig_sb[:, :, 576:578],
                    em_sb[:, None, 2:4].to_broadcast((128, 8, 2)), OP.mult)

        def proj_T(W_sb, Xt_sb, bt_sb, T_sb, pfx):
            """xT = (X @ W + b).T for program channels [0,512) -> head tiles."""
            for ct in range(4):
                for q2 in range(2):
                    ps = psA.tile([128, 512], F32, tag="mm", bufs=2,
                                  name=f"psT_{pfx}_{ct}_{q2}")
                    for dm in range(8):
                        nc.tensor.matmul(
                            ps,
                            W_sb[:, dm, ct * 128:ct * 128 + 128],
                            Xt_sb[:, dm, q2 * 512:q2 * 512 + 512],
                            start=(dm == 0), stop=(dm == 7))
                    nc.scalar.activation(
                        T_sb[:, ct, q2 * 512:q2 * 512 + 512], ps,
                        AF.Identity, bias=bt_sb[:, ct:ct + 1], scale=1.0)

        def conv_quarter(sig_sb, cw_d, T_sb, pfx, ldq, quarter, cpq=None):
            """One o'-quarter of: T[c,l] += conv output, scrambled back into
            head-transposed tiles.
            conv psum tile s: partition o' = 128 s + 64 ph + k, free l2' = 64 hl + m."""
            Tr = T_sb.rearrange("p q (m r) -> p q m r", r=16)
            if True:
                pss = [psA.tile([128, 512], F32, tag="conv", bufs=3,
                                name=f"cps_{pfx}_{quarter}_{si}") for si in range(2)]
                tres = None

                def emit_tres(si):
                    # residual: tres[o', l2'] = sig[d=l2', t=o'] via PE transpose
                    sblk = 2 * quarter + si
                    for lb2 in range(4):
                        nc.tensor.matmul(
                            tres[si][:, 128 * lb2:128 * lb2 + 128],
                            sig_sb[:, lb2, 64 + 128 * sblk:64 + 128 * sblk + 128],
                            ident_sb, is_transpose=True,
                            start=(lb2 == 0), stop=(lb2 == 3))

                if use_tres:
                    tres = [psA.tile([128, 512], BF16, tag="tres", bufs=3,
                                     name=f"tres_{pfx}_{quarter}_{si}") for si in range(2)]
                    emit_tres(0)

                cwts = []

                def cps_mm(si, dt):
                    for f in range(flen):
                        nc.tensor.matmul(
                            pss[si],
                            cwts[dt >> 1][:, dt & 1,
                                          f * 256 + si * 128:f * 256 + si * 128 + 128],
                            sig_sb[:, dt, 62 + f:62 + f + 512],
                            start=(dt == 0 and f == 0),
                            stop=(dt == 7 and f == flen - 1))

                def emit_adds(si):
                    sblk = 2 * quarter + si
                    ps = pss[si].rearrange("p (q h m) -> p q h m", q=4, h=2)
                    for ph in range(2):
                        for pe in range(2):
                            dst = Tr[64 * pe:64 * pe + 64, :, :, 2 * sblk + ph]
                            if use_tres:
                                # hw: ops may read only ONE psum operand, and
                                # gpsimd cannot touch psum — so: DVE copy of
                                # the transposed residual, then in-place add
                                tr2 = tres[si].rearrange(
                                    "p (q h m) -> p q h m", q=4, h=2)
                                if cpq is None:
                                    nc.vector.tensor_copy(
                                        dst, tr2[64 * ph:64 * ph + 64, :, pe, :])
                                else:
                                    cpq.activation(
                                        dst, tr2[64 * ph:64 * ph + 64, :, pe, :],
                                        AF.Copy)
                                nc.vector.tensor_tensor(
                                    dst, ps[64 * ph:64 * ph + 64, :, pe, :], dst,
                                    OP.add)
                            else:
                                nc.vector.tensor_tensor(
                                    dst, ps[64 * ph:64 * ph + 64, :, pe, :], dst,
                                    OP.add)

                # cw weights arrive as dt-pair DMAs (partition-remapped from
                # the (dt, p, c) DRAM layout) to halve issue/HWDGE overhead
                qsl = slice(quarter * flen * 256, (quarter + 1) * flen * 256)

                def load_pair(d2):
                    cwt = sb.tile([128, 2, flen * 256], BF16, tag="cw", bufs=6,
                                  name=f"cw_{pfx}_{quarter}_{d2}")
                    ldq.dma_start(cwt, cw_d[2 * d2:2 * d2 + 2, :, qsl].rearrange("dt p c -> p dt c"))
                    cwts.append(cwt)

                # si=0 leads by four dt-blocks so si=1's psum-slot wait and
                # tres si=1's are absorbed by real work; si=0 stops (and its
                # adds are emitted) before si=1's tail
                for d2 in range(2):
                    load_pair(d2)
                    cps_mm(0, 2 * d2)
                    cps_mm(0, 2 * d2 + 1)
                for dt in range(4):
                    cps_mm(1, dt)
                if use_tres:
                    emit_tres(1)
                for d2 in range(2, 4):
                    load_pair(d2)
                    cps_mm(0, 2 * d2)
                    cps_mm(0, 2 * d2 + 1)
                emit_adds(0)
                for dt in range(4, 8):
                    cps_mm(1, dt)
                emit_adds(1)


        def vp_tiles(Vt_sb, Wv_sb, lbs):
            for lb in lbs:
                ps = psA.tile([128, 512], F32, tag="mm", bufs=2, name=f"psv_{lb}")
                if not zero_bias:
                    nc.tensor.matmul(ps, ones1_sb[0:1, :], bVn_sb[0:1, :],
                                     start=True, stop=False)
                for dm in range(8):
                    nc.tensor.matmul(ps, Vt_sb[:, dm, lb * 128:lb * 128 + 128],
                                     Wv_sb[:, dm, :],
                                     start=(zero_bias and dm == 0), stop=(dm == 7))
                nc.vector.tensor_copy(
                    vpa_r[:, lb, :, 0:64],
                    ps.rearrange("p (hh c) -> p hh c", hh=8))

        # ---- phase sequence --------------------------------------------
        proj_nat(Qt_sb, Wq_sb, bQn_sb, qp2_d, "q", nc.scalar)
        scramble(qp2_d, sigq_sb, "q", pool_share=True)
        if not use_tres:
            proj_T(Wq_sb, Qt_sb, bQt_sb, qT_sb, "q")

        # V-side loads (SP) before the cw streams; Vt's qkvt WAR is satisfied
        # by the time SP reaches it
        Vt_sb = sb.tile([128, 8, L], BF16, tag="qkvt", bufs=2, name="Vt_sb")
        nc.sync.dma_start(Vt_sb, Vt_d.rearrange("(a p) c -> p a c", p=128))
        Wv_sb = sb.tile([128, 8, 512], BF16, name="Wv_sb")
        nc.sync.dma_start(Wv_sb, Wv_d.rearrange("(a p) c -> p a c", p=128))
        Wfc_sb = sb.tile([128, 4, DM], BF16, name="Wfc_sb")
        nc.sync.dma_start(Wfc_sb, Wfc_d.rearrange("(t p) c -> p t c", p=128))

        sig_fixup(sigq_sb)
        # manual interleave: 4 proj_k tiles per conv_q quarter so the DVE
        # queue (st copies | conv adds) drains in execution order
        ktiles = [(cb, lb) for cb in range(2) for lb in range(8)]
        for seg in range(4):
            for cb, lb in ktiles[4 * seg:4 * seg + 4]:
                proj_tile(Kt_sb, Wk_sb, bKn_sb, kp2_d, "k", nc.scalar, cb, lb)
            if seg == 1:
                scramble(kp2_d, sigk_sb, "k", pool_share=False, dts=range(4))
            if seg == 3:
                scramble(kp2_d, sigk_sb, "k", pool_share=False, dts=range(4, 8))
            conv_quarter(sigq_sb, cwq_d, qT_sb, "q", nc.sync, seg)
        if not use_tres:
            proj_T(Wk_sb, Kt_sb, bKt_sb, kT_sb, "k")
        sig_fixup(sigk_sb)
        # vp tiles interleave with conv_k quarters (DVE: vpa copies | adds)
        for seg in range(4):
            vp_tiles(Vt_sb, Wv_sb, range(2 * seg, 2 * seg + 2))
            conv_quarter(sigk_sb, cwk_d, kT_sb, "k", nc.sync, seg)

        psA.release()
        psB = tc.alloc_tile_pool(name="psB", bufs=1, space="PSUM")

        # ---- attention + fc, interleaved by query half ------------------
        odv = out_d.rearrange("(lb p) c -> p lb c", p=128)

        def score_tiles(qb, p4, rng):
            """Emit score matmuls + exp for (kt2, pe) pairs in rng; fill pts."""
            pt_tiles = _pts[(qb, p4)]
            for idx in rng:
                kt2, pe = idx >> 1, idx & 1
                ps_st = psB.tile([128, 1024], F32, tag="st", bufs=2,
                                 name=f"st_{p4}_{qb}_{kt2}_{pe}")
                for h in range(2):
                    kt = 2 * kt2 + h
                    nc.tensor.matmul(
                        ps_st[:, 512 * h:512 * h + 512],
                        kT_sb[64 * pe:64 * pe + 64, p4, kt * 128:kt * 128 + 128],
                        qT_sb[64 * pe:64 * pe + 64, p4, qb * 512:qb * 512 + 512],
                        start=True, stop=True, tile_position=(64 * pe, 0))
                pt = sb.tile([128, 1024], BF16, tag="pt", bufs=9,
                             name=f"pt_{p4}_{qb}_{kt2}_{pe}")
                nc.scalar.activation(pt, ps_st, AF.Exp, scale=0.125)
                pt_tiles[pe][2 * kt2] = pt[:, 0:512]
                pt_tiles[pe][2 * kt2 + 1] = pt[:, 512:1024]

        def ctx_half(qb, p4, pe):
            pt_tiles = _pts[(qb, p4)]
            hl = 2 * p4 + pe
            ps_ctx = psB.tile([65, 512], F32, tag="mm2", bufs=3,
                              name=f"ctx_{p4}_{qb}_{pe}")
            for kt in range(8):
                nc.tensor.matmul(
                    ps_ctx, vpa_sb[:, kt, 65 * hl:65 * hl + 65],
                    pt_tiles[pe][kt], start=(kt == 0), stop=(kt == 7))
            rc = sb.tile([1, 512], F32, tag="recip", bufs=2,
                         name=f"rc_{p4}_{qb}_{pe}")
            nc.vector.reciprocal(rc, ps_ctx[64:65, :])
            rcb = sb.tile([1, 512], BF16, tag="recipb", bufs=2,
                          name=f"rcb_{p4}_{qb}_{pe}")
            nc.vector.tensor_copy(rcb, rc)
            ps_bc = psB.tile([64, 512], F32, tag="bc", bufs=1,
                             name=f"bc_{p4}_{qb}_{pe}")
            nc.tensor.matmul(ps_bc, ones1_sb[0:1, 0:64], rcb,
                             start=True, stop=True)
            bc_sb = sb.tile([64, 512], BF16, tag="bcs", bufs=2,
                            name=f"bcs_{p4}_{qb}_{pe}")
            nc.vector.tensor_copy(bc_sb, ps_bc)
            nc.vector.tensor_tensor(
                ctxT_sb[64 * pe:64 * pe + 64, p4, qb * 512:qb * 512 + 512],
                ps_ctx[0:64, :], bc_sb, OP.mult)

        def fc_tile(lb, db):
            ps = psB.tile([128, 512], F32, tag="mm2", bufs=3,
                          name=f"fc_{lb}_{db}")
            for t4 in range(4):
                nc.tensor.matmul(
                    ps, ctxT_sb[:, t4, lb * 128:lb * 128 + 128],
                    Wfc_sb[:, t4, db * 512:db * 512 + 512],
                    start=(t4 == 0), stop=(t4 == 3))
            st = sb.tile([128, 512], BF16, tag="ostage", bufs=4,
                         name=f"ost_{lb}_{db}")
            nc.vector.tensor_copy(st, ps)
            nc.sync.dma_start(odv[:, lb, db * 512:db * 512 + 512], st)

        # software-pipelined attention: group g's ctx/bc/fc interleaves with
        # group g+1's score matmuls so the Act exp stream never starves
        groups = [(0, p4) for p4 in range(4)] + [(1, p4) for p4 in range(4)]
        _pts = {g: [[None] * 8 for _ in range(2)] for g in groups}
        # fc tiles assigned to groups: qb1 group p4 -> lb=p4; tail lbs 4-7
        fc_jobs = {g: [] for g in groups}
        for p4 in range(4):
            fc_jobs[(1, p4)] = [(p4, 0), (p4, 1)]
        score_tiles(*groups[0], range(8))
        for i, g in enumerate(groups):
            nxt = groups[i + 1] if i + 1 < len(groups) else None
            ctx_half(*g, 0)
            if nxt:
                score_tiles(*nxt, range(0, 4))
            ctx_half(*g, 1)
            if nxt:
                score_tiles(*nxt, range(4, 8))
            for lb, db in fc_jobs[g]:
                fc_tile(lb, db)
        for lb in range(4, 8):
            for db in range(2):
                fc_tile(lb, db)

        psB.release()
        sb.release()
        dr.release()

    nc.finalize()
    return nc


# ----------------------------------------------------------------------------
# host-side data prep
# ----------------------------------------------------------------------------
def _host_prep(inp, flen):
    """Build the 8 per-core input dicts (core ci = 2*b + g)."""
    # per-parity shared tensors (g = 0, 1)
    shared = []
    for g in range(2):
        pi = np.arange(DM) ^ (512 * g)
        d = {}
        d["WQ"] = np.ascontiguousarray(inp["WQ"][:, pi]).astype(bf16)
        d["WK"] = np.ascontiguousarray(inp["WK"][:, pi]).astype(bf16)
        d["WV"] = np.ascontiguousarray(inp["WV"][:, 512 * g:512 * g + 512]).astype(bf16)
        d["Wfc"] = np.ascontiguousarray(inp["Wfc"][512 * g:512 * g + 512, :]).astype(bf16)
        bQ = inp["bQ"][pi].astype(np.float32)
        bK = inp["bK"][pi].astype(np.float32)
        bV = inp["bV"][512 * g:512 * g + 512].astype(np.float32)
        d["bQn"] = bQ[None, :].astype(bf16)
        d["bKn"] = bK[None, :].astype(bf16)
        d["bVn"] = bV[None, :].astype(bf16)
        d["bQt"] = np.ascontiguousarray(bQ[:512].reshape(4, 128).T).astype(np.float32)
        d["bKt"] = np.ascontiguousarray(bK[:512].reshape(4, 128).T).astype(np.float32)
        for name, key in (("cwq", "conv_q"), ("cwk", "conv_k")):
            c = np.asarray(inp[key])[:, :, :flen].astype(np.float32)  # (d, o, f)
            c = np.ascontiguousarray(c.transpose(2, 0, 1))            # (f, d, o)
            c = c[:, pi, :][:, :, pi]
            # layout (8 dt, 128 p, 4 quarter, flen f, 256): column grouping so
            # each conv pass loads only its own o'-quarter of the weights
            c = c.transpose(1, 0, 2).reshape(8, 128, flen, 4, 256)
            c = np.ascontiguousarray(c.transpose(0, 1, 3, 2, 4)).reshape(8, 128, flen * 1024)
            d[name] = c.astype(bf16)
        em = np.zeros((128, 4), np.float32)
        em[:, :] = np.array([0, 0, 1, 1], np.float32) if g == 0 else \
            np.array([1, 1, 0, 0], np.float32)
        d["emask"] = em
        shared.append(d)

    maps = []
    for b in range(B):
        for g in range(2):
            sigma = np.arange(L) ^ (8 * g)
            m = dict(shared[g])
            m["Qt"] = np.ascontiguousarray(np.asarray(inp["Q"])[b][sigma, :].T).astype(bf16)
            m["Kt"] = np.ascontiguousarray(np.asarray(inp["K"])[b][sigma, :].T).astype(bf16)
            m["Vt"] = np.ascontiguousarray(np.asarray(inp["V"])[b][sigma, :].T).astype(bf16)
            maps.append(m)
    return maps


def _combine(results, inp):
    out = np.zeros((B, L, DM), np.float32)
    for b in range(B):
        for g in range(2):
            sigma = np.arange(L) ^ (8 * g)
            out[b] += np.asarray(results[2 * b + g]["out"])[sigma, :].astype(np.float32)
        out[b] += np.asarray(inp["bfc"], dtype=np.float32)
    return out


def _get_program(flen, zero_bias=False):
    key = (flen, zero_bias)
    if key not in _CACHE:
        _CACHE[key] = _build(flen, zero_bias=zero_bias)
    return _CACHE[key]


def run_on_cores(inputs, trace=False):
    """Run the SPMD kernel; returns (full_output, BassKernelResults)."""
    from concourse.bass_utils import run_bass_kernel_spmd
    inp = {k: np.asarray(v) for k, v in inputs.items()}
    f_s = np.array(FILTER_LENGTHS, np.float32)
    flen = int(FILTER_LENGTHS[int(np.argmax(f_s * np.asarray(inp["w"], np.float32)))])
    zb = all(not np.any(np.asarray(inp[k])) for k in ("bQ", "bK", "bV"))
    nc = _get_program(flen, zero_bias=zb)
    in_maps = _host_prep(inp, flen)
    res = run_bass_kernel_spmd(nc, in_maps, list(range(N_CORES)), trace=trace)
    return _combine(res.results, inp), res


def kernel(**inputs) -> np.ndarray:
    out, _ = run_on_cores(inputs, trace=False)
    return out


# revision 44
# speedup vs baseline: 1.0040x; 1.0040x over previous
# Trainium2 Bass kernel for nn_MultiHeadAttention_29154238005976 (ACAT-style conv-augmented MHA).
#
# Reference computation (B=4, L=1024, D=1024, H=16, DK=64):
#   q/k/v projections; q,k augmented by a "scrambled" depthwise-dense conv
#   (torch-style raw reshapes (b,h,l,dk)->(b, h*dk, l) scramble time/channels);
#   softmax attention per head; output projection.
#
# Sharding: 8 cores = 4 batches x 2 head-halves. All cores run an IDENTICAL
# program; per-core differences are pushed into host-side data permutations:
#   - time permutation sigma(l) = l XOR 8g applied to Q/K/V rows (g = head-half)
#   - channel permutation pi(c) = c XOR 512g applied to WQ/WK columns and to
#     conv kernel rows+columns
#   - conv "time" halves then both map to program range l2' in [0,512)
#   - boundary sig columns get a per-core 0/1 mask (true zero-padding vs
#     wrapped real data)
# Host gathers the 8 partial outputs, un-permutes rows, sums batch pairs, +bfc.
#
# Scramble path: the projection psum tiles are copied (f32->bf16) to SBUF
# staging tiles, staged to DRAM in a gather-friendly layout
# qp2[jh, il, ihi, jl]  (qp row l = 16*il + jh, col c = 64*ihi + jl),
# so each sig gather (dt, hib) is a 3-dim DMA and the wrap block needs no
# DMA at all for flen=2 (only cols 62:64 are ever read; they equal cols
# 1086:1088 masked by em).
import numpy as np
import ml_dtypes

import concourse.bass as bass
import concourse.mybir as mybir
import concourse.tile as tile
from concourse import bacc

bf16 = ml_dtypes.bfloat16
F32 = mybir.dt.float32
BF16 = mybir.dt.bfloat16
AF = mybir.ActivationFunctionType
OP = mybir.AluOpType

B, L, DM, H, DK = 4, 1024, 1024, 16, 64
FMAX = 4
FILTER_LENGTHS = (2, 4)
N_CORES = 8

_CACHE = {}


# ----------------------------------------------------------------------------
# program builder
# ----------------------------------------------------------------------------
def _build(flen: int, zero_bias: bool = False) -> bass.Bass:
    use_tres = (flen == 2)  # residual via PE-transpose folded into conv psum
    nc = bacc.Bacc("TRN2", target_bir_lowering=False, debug=False)

    def din(name, shape, dt):
        return nc.dram_tensor(name, list(shape), dt, kind="ExternalInput").ap()

    Qt_d = din("Qt", (DM, L), BF16)
    Kt_d = din("Kt", (DM, L), BF16)
    Vt_d = din("Vt", (DM, L), BF16)
    Wq_d = din("WQ", (DM, DM), BF16)
    Wk_d = din("WK", (DM, DM), BF16)
    Wv_d = din("WV", (DM, 512), BF16)
    Wfc_d = din("Wfc", (512, DM), BF16)
    cwq_d = din("cwq", (8, 128, flen * 1024), BF16)
    cwk_d = din("cwk", (8, 128, flen * 1024), BF16)
    bQn_d = din("bQn", (1, DM), BF16)
    bKn_d = din("bKn", (1, DM), BF16)
    bVn_d = din("bVn", (1, 512), BF16)
    bQt_d = din("bQt", (128, 4), F32)
    bKt_d = din("bKt", (128, 4), F32)
    em_d = din("emask", (128, 4), F32)
    out_d = nc.dram_tensor("out", [L, DM], BF16, kind="ExternalOutput").ap()

    njh = 16 if use_tres else 9        # gathered 64-col j-bands per (dt, hib)
    SIGW = 64 + 64 * njh               # 1088 (flen=2) / 640 (flen=4)

    with tile.TileContext(nc) as tc:
        sb = tc.alloc_tile_pool(name="sb", bufs=1)
        dr = tc.alloc_tile_pool(name="dr", bufs=1, space="DRAM")
        psA = tc.alloc_tile_pool(name="psA", bufs=1, space="PSUM")

        # ---- tiny setup ------------------------------------------------
        em_sb = sb.tile([128, 4], F32, name="em_sb")
        ones1_sb = sb.tile([1, 128], BF16, name="ones1_sb")
        nc.vector.memset(ones1_sb, 1.0)
        if not zero_bias:
            bQn_sb = sb.tile([1, DM], BF16, name="bQn_sb")
            nc.sync.dma_start(bQn_sb, bQn_d)
            bKn_sb = sb.tile([1, DM], BF16, name="bKn_sb")
            nc.sync.dma_start(bKn_sb, bKn_d)
            bVn_sb = sb.tile([1, 512], BF16, name="bVn_sb")
            nc.sync.dma_start(bVn_sb, bVn_d)
        else:
            bQn_sb = bKn_sb = bVn_sb = None
        if not use_tres:
            bQt_sb = sb.tile([128, 4], F32, name="bQt_sb")
            nc.sync.dma_start(bQt_sb, bQt_d)
            bKt_sb = sb.tile([128, 4], F32, name="bKt_sb")
            nc.sync.dma_start(bKt_sb, bKt_d)

        # ---- big loads: first-use order, split across SP and Pool ------
        # SP/HWDGE carries Qt, Kt, Wq(cb1), Wk, Vt, Wv, Wfc, cw.
        # Pool/SWDGE carries Wq(cb0) so the first matmuls aren't serialized
        # behind the SP issue queue.
        Qt_sb = sb.tile([128, 8, L], BF16, tag="qkvt", bufs=2, name="Qt_sb")
        Wq_sb = sb.tile([128, 8, DM], BF16, name="Wq_sb")
        qtv = Qt_d.rearrange("(a p) c -> p a c", p=128)
        wqv = Wq_d.rearrange("(a p) c -> p a c", p=128)
        nc.sync.dma_start(Qt_sb[:, 0, 0:128], qtv[:, 0, 0:128])
        nc.sync.dma_start(Qt_sb[:, 0, 128:1024], qtv[:, 0, 128:1024])
        nc.sync.dma_start(em_sb, em_d)
        for a in range(8):
            if a % 2 == 0:
                nc.gpsimd.dma_start(Wq_sb[:, a, 0:512], wqv[:, a, 0:512])
            if a > 0:
                nc.sync.dma_start(Qt_sb[:, a], qtv[:, a])
        for a in range(1, 8, 2):
            nc.sync.dma_start(Wq_sb[:, a, 0:512], wqv[:, a, 0:512])
        Wk_sb = sb.tile([128, 8, DM], BF16, name="Wk_sb")
        Kt_sb = sb.tile([128, 8, L], BF16, tag="qkvt", bufs=2, name="Kt_sb")
        wkv = Wk_d.rearrange("(a p) c -> p a c", p=128)
        ktv = Kt_d.rearrange("(a p) c -> p a c", p=128)
        for a in range(8):
            nc.gpsimd.dma_start(Wk_sb[:, a, 0:512], wkv[:, a, 0:512])
            nc.sync.dma_start(Wq_sb[:, a, 512:1024], wqv[:, a, 512:1024])
            nc.sync.dma_start(Kt_sb[:, a], ktv[:, a])
            nc.sync.dma_start(Wk_sb[:, a, 512:1024], wkv[:, a, 512:1024])

        qT_sb = sb.tile([128, 4, L], BF16, name="qT_sb")
        kT_sb = sb.tile([128, 4, L], BF16, name="kT_sb")
        ctxT_sb = sb.tile([128, 4, L], BF16, name="ctxT_sb")
        sigq_sb = sb.tile([128, 8, SIGW], BF16, name="sigq_sb")
        sigk_sb = sb.tile([128, 8, SIGW], BF16, name="sigk_sb")
        if use_tres:
            from concourse.masks import make_identity
            ident_sb = sb.tile([128, 128], BF16, name="ident_sb")
            make_identity(nc, ident_sb)
        vpa_sb = sb.tile([128, 8, 520], BF16, name="vpa_sb")
        vpa_r = vpa_sb.rearrange("p lb (hh c) -> p lb hh c", c=65)
        nc.vector.memset(vpa_r[:, :, :, 64], 1.0)  # the denominator "ones" column

        # staging DRAM in scramble layout [jh, il, ihi, jl]
        qp2_d = dr.tile([16, 64, 16, 64], BF16, name="qp2_d")
        kp2_d = dr.tile([16, 64, 16, 64], BF16, name="kp2_d")

        # ---- phase helpers ---------------------------------------------
        def proj_tile(Xt_sb, W_sb, bn_sb, x2_d, pfx, wq, cb, lb):
            """One l/c tile of x = X @ W (+b), staged to DRAM in scramble layout."""
            x2w = x2_d.rearrange("jh il ihi jl -> il jh (ihi jl)")
            ps = psA.tile([128, 512], F32, tag="mm", bufs=2,
                          name=f"ps_{pfx}_{cb}_{lb}")
            if not zero_bias:
                nc.tensor.matmul(ps, ones1_sb[0:1, :],
                                 bn_sb[0:1, cb * 512:cb * 512 + 512],
                                 start=True, stop=False)
            for dm in range(8):
                nc.tensor.matmul(
                    ps,
                    Xt_sb[:, dm, lb * 128:lb * 128 + 128],
                    W_sb[:, dm, cb * 512:cb * 512 + 512],
                    start=(zero_bias and dm == 0), stop=(dm == 7))
            st = sb.tile([128, 512], BF16, tag="stage", bufs=10,
                         name=f"st_{pfx}_{cb}_{lb}")
            nc.vector.tensor_copy(st, ps)
            # st partition p = 16*il2 + jh iterates in the same linear
            # order as the dst view's (il2, jh) dims
            wq.dma_start(
                x2w[8 * lb:8 * lb + 8, :, 512 * cb:512 * cb + 512], st)

        def proj_nat(Xt_sb, W_sb, bn_sb, x2_d, pfx, wq):
            for cb in range(2):
                for lb in range(8):
                    proj_tile(Xt_sb, W_sb, bn_sb, x2_d, pfx, wq, cb, lb)

        def scramble(x2_d, sig_sb, pfx, pool_share, dts=range(8)):
            """sig[64 hib + il, dt, 64 + 64 jh + jl] <- qp[16 il + jh, 64 ihi + jl]
            (ihi = 2 dt + hib); boundary cols 62:64 fixed up separately."""
            x2g = x2_d.rearrange("jh il ihi jl -> il jh ihi jl")
            for dt in dts:
                for hib in range(2):
                    ihi = 2 * dt + hib
                    dst = sig_sb[64 * hib:64 * hib + 64, dt, 64:64 + 64 * njh]
                    q = nc.gpsimd if (pool_share and dt < 4) else nc.scalar
                    q.dma_start(
                        dst.rearrange("p (jh jl) -> p jh jl", jl=64),
                        x2g[:, 0:njh, ihi])
                    if not use_tres:
                        # flen=4: cols 62:64 (j = 1022:1024) lie outside the
                        # gathered bands; fetch the 2-col wrap strip directly.
                        nc.scalar.dma_start(sig_sb[64 * hib:64 * hib + 64, dt, 62:64],
                                            x2g[:, 15, ihi, 62:64])

        def sig_fixup(sig_sb):
            if use_tres:
                # wrap cols 62:64 duplicate cols 1086:1088; apply pad mask em
                nc.gpsimd.tensor_tensor(
                    sig_sb[:, :, 62:64], sig_sb[:, :, 1086:1088],
                    em_sb[:, None, 0:2].to_broadcast((128, 8, 2)), OP.mult)
            else:
                nc.gpsimd.tensor_tensor(
                    sig_sb[:, :, 62:64], sig_sb[:, :, 62:64],
                    em_sb[:, None, 0:2].to_broadcast((128, 8, 2)), OP.mult)
                nc.gpsimd.tensor_tensor(
                    sig_sb[:, :, 576:578], sig_sb[:, :, 576:578],
                    em_sb[:, None, 2:4].to_broadcast((128, 8, 2)), OP.mult)

        def proj_T(W_sb, Xt_sb, bt_sb, T_sb, pfx):
            """xT = (X @ W + b).T for program channels [0,512) -> head tiles."""
            for ct in range(4):
                for q2 in range(2):
                    ps = psA.tile([128, 512], F32, tag="mm", bufs=2,
                                  name=f"psT_{pfx}_{ct}_{q2}")
                    for dm in range(8):
                        nc.tensor.matmul(
                            ps,
                            W_sb[:, dm, ct * 128:ct * 128 + 128],
                            Xt_sb[:, dm, q2 * 512:q2 * 512 + 512],
                            start=(dm == 0), stop=(dm == 7))
                    nc.scalar.activation(
                        T_sb[:, ct, q2 * 512:q2 * 512 + 512], ps,
                        AF.Identity, bias=bt_sb[:, ct:ct + 1], scale=1.0)

        def conv_quarter(sig_sb, cw_d, T_sb, pfx, ldq, quarter, cpq=None):
            """One o'-quarter of: T[c,l] += conv output, scrambled back into
            head-transposed tiles.
            conv psum tile s: partition o' = 128 s + 64 ph + k, free l2' = 64 hl + m."""
            Tr = T_sb.rearrange("p q (m r) -> p q m r", r=16)
            if True:
                pss = [psA.tile([128, 512], F32, tag="conv", bufs=3,
                                name=f"cps_{pfx}_{quarter}_{si}") for si in range(2)]
                tres = None

                def emit_tres(si):
                    # residual: tres[o', l2'] = sig[d=l2', t=o'] via PE transpose
                    sblk = 2 * quarter + si
                    for lb2 in range(4):
                        nc.tensor.matmul(
                            tres[si][:, 128 * lb2:128 * lb2 + 128],
                            sig_sb[:, lb2, 64 + 128 * sblk:64 + 128 * sblk + 128],
                            ident_sb, is_transpose=True,
                            start=(lb2 == 0), stop=(lb2 == 3))

                if use_tres:
                    tres = [psA.tile([128, 512], BF16, tag="tres", bufs=3,
                                     name=f"tres_{pfx}_{quarter}_{si}") for si in range(2)]
                    emit_tres(0)

                cwts = []

                def cps_mm(si, dt):
                    for f in range(flen):
                        nc.tensor.matmul(
                            pss[si],
                            cwts[dt >> 1][:, dt & 1,
                                          f * 256 + si * 128:f * 256 + si * 128 + 128],
                            sig_sb[:, dt, 62 + f:62 + f + 512],
                            start=(dt == 0 and f == 0),
                            stop=(dt == 7 and f == flen - 1))

                def emit_adds(si):
                    sblk = 2 * quarter + si
                    ps = pss[si].rearrange("p (q h m) -> p q h m", q=4, h=2)
                    for ph in range(2):
                        for pe in range(2):
                            dst = Tr[64 * pe:64 * pe + 64, :, :, 2 * sblk + ph]
                            if use_tres:
                                # hw: ops may read only ONE psum operand, and
                                # gpsimd cannot touch psum — so: DVE copy of
                                # the transposed residual, then in-place add
                                tr2 = tres[si].rearrange(
                                    "p (q h m) -> p q h m", q=4, h=2)
                                if cpq is None:
                                    nc.vector.tensor_copy(
                                        dst, tr2[64 * ph:64 * ph + 64, :, pe, :])
                                else:
                                    cpq.activation(
                                        dst, tr2[64 * ph:64 * ph + 64, :, pe, :],
                                        AF.Copy)
                                nc.vector.tensor_tensor(
                                    dst, ps[64 * ph:64 * ph + 64, :, pe, :], dst,
                                    OP.add)
                            else:
                                nc.vector.tensor_tensor(
                                    dst, ps[64 * ph:64 * ph + 64, :, pe, :], dst,
                                    OP.add)

                # cw weights arrive as dt-pair DMAs (partition-remapped from
                # the (dt, p, c) DRAM layout) to halve issue/HWDGE overhead
                qsl = slice(quarter * flen * 256, (quarter + 1) * flen * 256)

                def load_pair(d2):
                    cwt = sb.tile([128, 2, flen * 256], BF16, tag="cw", bufs=5,
                                  name=f"cw_{pfx}_{quarter}_{d2}")
                    ldq.dma_start(cwt, cw_d[2 * d2:2 * d2 + 2, :, qsl].rearrange("dt p c -> p dt c"))
                    cwts.append(cwt)

                # si=0 leads by four dt-blocks so si=1's psum-slot wait and
                # tres si=1's are absorbed by real work; si=0 stops (and its
                # adds are emitted) before si=1's tail
                for d2 in range(2):
                    load_pair(d2)
                    cps_mm(0, 2 * d2)
                    cps_mm(0, 2 * d2 + 1)
                for dt in range(4):
                    cps_mm(1, dt)
                if use_tres:
                    emit_tres(1)
                for d2 in range(2, 4):
                    load_pair(d2)
                    cps_mm(0, 2 * d2)
                    cps_mm(0, 2 * d2 + 1)
                emit_adds(0)
                for dt in range(4, 8):
                    cps_mm(1, dt)
                emit_adds(1)


        def vp_tiles(Vt_sb, Wv_sb, lbs):
            for lb in lbs:
                ps = psA.tile([128, 512], F32, tag="mm", bufs=2, name=f"psv_{lb}")
                if not zero_bias:
                    nc.tensor.matmul(ps, ones1_sb[0:1, :], bVn_sb[0:1, :],
                                     start=True, stop=False)
                for dm in range(8):
                    nc.tensor.matmul(ps, Vt_sb[:, dm, lb * 128:lb * 128 + 128],
                                     Wv_sb[:, dm, :],
                                     start=(zero_bias and dm == 0), stop=(dm == 7))
                nc.vector.tensor_copy(
                    vpa_r[:, lb, :, 0:64],
                    ps.rearrange("p (hh c) -> p hh c", hh=8))

        # ---- phase sequence --------------------------------------------
        proj_nat(Qt_sb, Wq_sb, bQn_sb, qp2_d, "q", nc.scalar)
        scramble(qp2_d, sigq_sb, "q", pool_share=True)
        if not use_tres:
            proj_T(Wq_sb, Qt_sb, bQt_sb, qT_sb, "q")

        # V-side loads (SP) before the cw streams; Vt's qkvt WAR is satisfied
        # by the time SP reaches it
        Vt_sb = sb.tile([128, 8, L], BF16, tag="qkvt", bufs=2, name="Vt_sb")
        nc.sync.dma_start(Vt_sb, Vt_d.rearrange("(a p) c -> p a c", p=128))
        Wv_sb = sb.tile([128, 8, 512], BF16, name="Wv_sb")
        nc.sync.dma_start(Wv_sb, Wv_d.rearrange("(a p) c -> p a c", p=128))
        Wfc_sb = sb.tile([128, 4, DM], BF16, name="Wfc_sb")
        nc.sync.dma_start(Wfc_sb, Wfc_d.rearrange("(t p) c -> p t c", p=128))

        sig_fixup(sigq_sb)
        # manual interleave: 4 proj_k tiles per conv_q quarter so the DVE
        # queue (st copies | conv adds) drains in execution order
        ktiles = [(cb, lb) for cb in range(2) for lb in range(8)]
        for seg in range(4):
            for cb, lb in ktiles[4 * seg:4 * seg + 4]:
                proj_tile(Kt_sb, Wk_sb, bKn_sb, kp2_d, "k", nc.scalar, cb, lb)
            if seg == 1:
                scramble(kp2_d, sigk_sb, "k", pool_share=False, dts=range(4))
            if seg == 3:
                scramble(kp2_d, sigk_sb, "k", pool_share=False, dts=range(4, 8))
            conv_quarter(sigq_sb, cwq_d, qT_sb, "q", nc.sync, seg)
        if not use_tres:
            proj_T(Wk_sb, Kt_sb, bKt_sb, kT_sb, "k")
        sig_fixup(sigk_sb)
        # vp tiles interleave with conv_k quarters (DVE: vpa copies | adds)
        for seg in range(4):
            vp_tiles(Vt_sb, Wv_sb, range(2 * seg, 2 * seg + 2))
            conv_quarter(sigk_sb, cwk_d, kT_sb, "k", nc.sync, seg)

        psA.release()
        psB = tc.alloc_tile_pool(name="psB", bufs=1, space="PSUM")

        # ---- attention + fc, interleaved by query half ------------------
        odv = out_d.rearrange("(lb p) c -> p lb c", p=128)

        def score_tiles(qb, p4, rng):
            """Emit score matmuls + exp for (kt2, pe) pairs in rng; fill pts."""
            pt_tiles = _pts[(qb, p4)]
            for idx in rng:
                kt2, pe = idx >> 1, idx & 1
                ps_st = psB.tile([128, 1024], F32, tag="st", bufs=2,
                                 name=f"st_{p4}_{qb}_{kt2}_{pe}")
                for h in range(2):
                    kt = 2 * kt2 + h
                    nc.tensor.matmul(
                        ps_st[:, 512 * h:512 * h + 512],
                        kT_sb[64 * pe:64 * pe + 64, p4, kt * 128:kt * 128 + 128],
                        qT_sb[64 * pe:64 * pe + 64, p4, qb * 512:qb * 512 + 512],
                        start=True, stop=True, tile_position=(64 * pe, 0))
                pt = sb.tile([128, 1024], BF16, tag="pt", bufs=9,
                             name=f"pt_{p4}_{qb}_{kt2}_{pe}")
                nc.scalar.activation(pt, ps_st, AF.Exp, scale=0.125)
                pt_tiles[pe][2 * kt2] = pt[:, 0:512]
                pt_tiles[pe][2 * kt2 + 1] = pt[:, 512:1024]

        def ctx_half(qb, p4, pe):
            pt_tiles = _pts[(qb, p4)]
            hl = 2 * p4 + pe
            ps_ctx = psB.tile([65, 512], F32, tag="mm2", bufs=3,
                              name=f"ctx_{p4}_{qb}_{pe}")
            for kt in range(8):
                nc.tensor.matmul(
                    ps_ctx, vpa_sb[:, kt, 65 * hl:65 * hl + 65],
                    pt_tiles[pe][kt], start=(kt == 0), stop=(kt == 7))
            rc = sb.tile([1, 512], F32, tag="recip", bufs=2,
                         name=f"rc_{p4}_{qb}_{pe}")
            nc.vector.reciprocal(rc, ps_ctx[64:65, :])
            rcb = sb.tile([1, 512], BF16, tag="recipb", bufs=2,
                          name=f"rcb_{p4}_{qb}_{pe}")
            nc.vector.tensor_copy(rcb, rc)
            ps_bc = psB.tile([64, 512], F32, tag="bc", bufs=1,
                             name=f"bc_{p4}_{qb}_{pe}")
            nc.tensor.matmul(ps_bc, ones1_sb[0:1, 0:64], rcb,
                             start=True, stop=True)
            bc_sb = sb.tile([64, 512], BF16, tag="bcs", bufs=2,
                            name=f"bcs_{p4}_{qb}_{pe}")
            nc.vector.tensor_copy(bc_sb, ps_bc)
            nc.vector.tensor_tensor(
                ctxT_sb[64 * pe:64 * pe + 64, p4, qb * 512:qb * 512 + 512],
                ps_ctx[0:64, :], bc_sb, OP.mult)

        def fc_tile(lb, db):
            ps = psB.tile([128, 512], F32, tag="mm2", bufs=3,
                          name=f"fc_{lb}_{db}")
            for t4 in range(4):
                nc.tensor.matmul(
                    ps, ctxT_sb[:, t4, lb * 128:lb * 128 + 128],
                    Wfc_sb[:, t4, db * 512:db * 512 + 512],
                    start=(t4 == 0), stop=(t4 == 3))
            st = sb.tile([128, 512], BF16, tag="ostage", bufs=4,
                         name=f"ost_{lb}_{db}")
            nc.vector.tensor_copy(st, ps)
            nc.sync.dma_start(odv[:, lb, db * 512:db * 512 + 512], st)

        # software-pipelined attention: group g's ctx/bc/fc interleaves with
        # group g+1's score matmuls so the Act exp stream never starves
        groups = [(0, p4) for p4 in range(4)] + [(1, p4) for p4 in range(4)]
        _pts = {g: [[None] * 8 for _ in range(2)] for g in groups}
        # fc tiles assigned to groups: qb1 group p4 -> lb=p4; tail lbs 4-7
        fc_jobs = {g: [] for g in groups}
        for p4 in range(4):
            fc_jobs[(1, p4)] = [(p4, 0), (p4, 1)]
        score_tiles(*groups[0], range(8))
        for i, g in enumerate(groups):
            nxt = groups[i + 1] if i + 1 < len(groups) else None
            ctx_half(*g, 0)
            if nxt:
                score_tiles(*nxt, range(0, 4))
            ctx_half(*g, 1)
            if nxt:
                score_tiles(*nxt, range(4, 8))
            for lb, db in fc_jobs[g]:
                fc_tile(lb, db)
        for lb in range(4, 8):
            for db in range(2):
                fc_tile(lb, db)

        psB.release()
        sb.release()
        dr.release()

    nc.finalize()
    return nc


# ----------------------------------------------------------------------------
# host-side data prep
# ----------------------------------------------------------------------------
def _host_prep(inp, flen):
    """Build the 8 per-core input dicts (core ci = 2*b + g)."""
    # per-parity shared tensors (g = 0, 1)
    shared = []
    for g in range(2):
        pi = np.arange(DM) ^ (512 * g)
        d = {}
        d["WQ"] = np.ascontiguousarray(inp["WQ"][:, pi]).astype(bf16)
        d["WK"] = np.ascontiguousarray(inp["WK"][:, pi]).astype(bf16)
        d["WV"] = np.ascontiguousarray(inp["WV"][:, 512 * g:512 * g + 512]).astype(bf16)
        d["Wfc"] = np.ascontiguousarray(inp["Wfc"][512 * g:512 * g + 512, :]).astype(bf16)
        bQ = inp["bQ"][pi].astype(np.float32)
        bK = inp["bK"][pi].astype(np.float32)
        bV = inp["bV"][512 * g:512 * g + 512].astype(np.float32)
        d["bQn"] = bQ[None, :].astype(bf16)
        d["bKn"] = bK[None, :].astype(bf16)
        d["bVn"] = bV[None, :].astype(bf16)
        d["bQt"] = np.ascontiguousarray(bQ[:512].reshape(4, 128).T).astype(np.float32)
        d["bKt"] = np.ascontiguousarray(bK[:512].reshape(4, 128).T).astype(np.float32)
        for name, key in (("cwq", "conv_q"), ("cwk", "conv_k")):
            c = np.asarray(inp[key])[:, :, :flen].astype(np.float32)  # (d, o, f)
            c = np.ascontiguousarray(c.transpose(2, 0, 1))            # (f, d, o)
            c = c[:, pi, :][:, :, pi]
            # layout (8 dt, 128 p, 4 quarter, flen f, 256): column grouping so
            # each conv pass loads only its own o'-quarter of the weights
            c = c.transpose(1, 0, 2).reshape(8, 128, flen, 4, 256)
            c = np.ascontiguousarray(c.transpose(0, 1, 3, 2, 4)).reshape(8, 128, flen * 1024)
            d[name] = c.astype(bf16)
        em = np.zeros((128, 4), np.float32)
        em[:, :] = np.array([0, 0, 1, 1], np.float32) if g == 0 else \
            np.array([1, 1, 0, 0], np.float32)
        d["emask"] = em
        shared.append(d)

    maps = []
    for b in range(B):
        for g in range(2):
            sigma = np.arange(L) ^ (8 * g)
            m = dict(shared[g])
            m["Qt"] = np.ascontiguousarray(np.asarray(inp["Q"])[b][sigma, :].T).astype(bf16)
            m["Kt"] = np.ascontiguousarray(np.asarray(inp["K"])[b][sigma, :].T).astype(bf16)
            m["Vt"] = np.ascontiguousarray(np.asarray(inp["V"])[b][sigma, :].T).astype(bf16)
            maps.append(m)
    return maps


def _combine(results, inp):
    out = np.zeros((B, L, DM), np.float32)
    for b in range(B):
        for g in range(2):
            sigma = np.arange(L) ^ (8 * g)
            out[b] += np.asarray(results[2 * b + g]["out"])[sigma, :].astype(np.float32)
        out[b] += np.asarray(inp["bfc"], dtype=np.float32)
    return out


def _get_program(flen, zero_bias=False):
    key = (flen, zero_bias)
    if key not in _CACHE:
        _CACHE[key] = _build(flen, zero_bias=zero_bias)
    return _CACHE[key]


def run_on_cores(inputs, trace=False):
    """Run the SPMD kernel; returns (full_output, BassKernelResults)."""
    from concourse.bass_utils import run_bass_kernel_spmd
    inp = {k: np.asarray(v) for k, v in inputs.items()}
    f_s = np.array(FILTER_LENGTHS, np.float32)
    flen = int(FILTER_LENGTHS[int(np.argmax(f_s * np.asarray(inp["w"], np.float32)))])
    zb = all(not np.any(np.asarray(inp[k])) for k in ("bQ", "bK", "bV"))
    nc = _get_program(flen, zero_bias=zb)
    in_maps = _host_prep(inp, flen)
    res = run_bass_kernel_spmd(nc, in_maps, list(range(N_CORES)), trace=trace)
    return _combine(res.results, inp), res


def kernel(**inputs) -> np.ndarray:
    out, _ = run_on_cores(inputs, trace=False)
    return out


# revision 45
# speedup vs baseline: 1.0059x; 1.0019x over previous
# Trainium2 Bass kernel for nn_MultiHeadAttention_29154238005976 (ACAT-style conv-augmented MHA).
#
# Reference computation (B=4, L=1024, D=1024, H=16, DK=64):
#   q/k/v projections; q,k augmented by a "scrambled" depthwise-dense conv
#   (torch-style raw reshapes (b,h,l,dk)->(b, h*dk, l) scramble time/channels);
#   softmax attention per head; output projection.
#
# Sharding: 8 cores = 4 batches x 2 head-halves. All cores run an IDENTICAL
# program; per-core differences are pushed into host-side data permutations:
#   - time permutation sigma(l) = l XOR 8g applied to Q/K/V rows (g = head-half)
#   - channel permutation pi(c) = c XOR 512g applied to WQ/WK columns and to
#     conv kernel rows+columns
#   - conv "time" halves then both map to program range l2' in [0,512)
#   - boundary sig columns get a per-core 0/1 mask (true zero-padding vs
#     wrapped real data)
# Host gathers the 8 partial outputs, un-permutes rows, sums batch pairs, +bfc.
#
# Scramble path: the projection psum tiles are copied (f32->bf16) to SBUF
# staging tiles, staged to DRAM in a gather-friendly layout
# qp2[jh, il, ihi, jl]  (qp row l = 16*il + jh, col c = 64*ihi + jl),
# so each sig gather (dt, hib) is a 3-dim DMA and the wrap block needs no
# DMA at all for flen=2 (only cols 62:64 are ever read; they equal cols
# 1086:1088 masked by em).
import numpy as np
import ml_dtypes

import concourse.bass as bass
import concourse.mybir as mybir
import concourse.tile as tile
from concourse import bacc

bf16 = ml_dtypes.bfloat16
F32 = mybir.dt.float32
BF16 = mybir.dt.bfloat16
AF = mybir.ActivationFunctionType
OP = mybir.AluOpType

B, L, DM, H, DK = 4, 1024, 1024, 16, 64
FMAX = 4
FILTER_LENGTHS = (2, 4)
N_CORES = 8

_CACHE = {}


# ----------------------------------------------------------------------------
# program builder
# ----------------------------------------------------------------------------
def _build(flen: int, zero_bias: bool = False) -> bass.Bass:
    use_tres = (flen == 2)  # residual via PE-transpose folded into conv psum
    nc = bacc.Bacc("TRN2", target_bir_lowering=False, debug=False)

    def din(name, shape, dt):
        return nc.dram_tensor(name, list(shape), dt, kind="ExternalInput").ap()

    Qt_d = din("Qt", (DM, L), BF16)
    Kt_d = din("Kt", (DM, L), BF16)
    Vt_d = din("Vt", (DM, L), BF16)
    Wq_d = din("WQ", (DM, DM), BF16)
    Wk_d = din("WK", (DM, DM), BF16)
    Wv_d = din("WV", (DM, 512), BF16)
    Wfc_d = din("Wfc", (512, DM), BF16)
    cwq_d = din("cwq", (8, 128, flen * 1024), BF16)
    cwk_d = din("cwk", (8, 128, flen * 1024), BF16)
    bQn_d = din("bQn", (1, DM), BF16)
    bKn_d = din("bKn", (1, DM), BF16)
    bVn_d = din("bVn", (1, 512), BF16)
    bQt_d = din("bQt", (128, 4), F32)
    bKt_d = din("bKt", (128, 4), F32)
    em_d = din("emask", (128, 4), F32)
    out_d = nc.dram_tensor("out", [L, DM], BF16, kind="ExternalOutput").ap()

    njh = 16 if use_tres else 9        # gathered 64-col j-bands per (dt, hib)
    SIGW = 64 + 64 * njh               # 1088 (flen=2) / 640 (flen=4)

    with tile.TileContext(nc) as tc:
        sb = tc.alloc_tile_pool(name="sb", bufs=1)
        dr = tc.alloc_tile_pool(name="dr", bufs=1, space="DRAM")
        psA = tc.alloc_tile_pool(name="psA", bufs=1, space="PSUM")

        # ---- tiny setup ------------------------------------------------
        em_sb = sb.tile([128, 4], F32, name="em_sb")
        ones1_sb = sb.tile([1, 128], BF16, name="ones1_sb")
        nc.vector.memset(ones1_sb, 1.0)
        if not zero_bias:
            bQn_sb = sb.tile([1, DM], BF16, name="bQn_sb")
            nc.sync.dma_start(bQn_sb, bQn_d)
            bKn_sb = sb.tile([1, DM], BF16, name="bKn_sb")
            nc.sync.dma_start(bKn_sb, bKn_d)
            bVn_sb = sb.tile([1, 512], BF16, name="bVn_sb")
            nc.sync.dma_start(bVn_sb, bVn_d)
        else:
            bQn_sb = bKn_sb = bVn_sb = None
        if not use_tres:
            bQt_sb = sb.tile([128, 4], F32, name="bQt_sb")
            nc.sync.dma_start(bQt_sb, bQt_d)
            bKt_sb = sb.tile([128, 4], F32, name="bKt_sb")
            nc.sync.dma_start(bKt_sb, bKt_d)

        # ---- big loads: first-use order, split across SP and Pool ------
        # SP/HWDGE carries Qt, Kt, Wq(cb1), Wk, Vt, Wv, Wfc, cw.
        # Pool/SWDGE carries Wq(cb0) so the first matmuls aren't serialized
        # behind the SP issue queue.
        Qt_sb = sb.tile([128, 8, L], BF16, tag="qkvt", bufs=2, name="Qt_sb")
        Wq_sb = sb.tile([128, 8, DM], BF16, name="Wq_sb")
        qtv = Qt_d.rearrange("(a p) c -> p a c", p=128)
        wqv = Wq_d.rearrange("(a p) c -> p a c", p=128)
        nc.sync.dma_start(Qt_sb[:, 0, 0:128], qtv[:, 0, 0:128])
        nc.sync.dma_start(Qt_sb[:, 0, 128:1024], qtv[:, 0, 128:1024])
        nc.sync.dma_start(em_sb, em_d)
        for a in range(8):
            if a % 2 == 0:
                nc.gpsimd.dma_start(Wq_sb[:, a, 0:512], wqv[:, a, 0:512])
            if a > 0:
                nc.sync.dma_start(Qt_sb[:, a], qtv[:, a])
        for a in range(1, 8, 2):
            nc.sync.dma_start(Wq_sb[:, a, 0:512], wqv[:, a, 0:512])
        Wk_sb = sb.tile([128, 8, DM], BF16, name="Wk_sb")
        Kt_sb = sb.tile([128, 8, L], BF16, tag="qkvt", bufs=2, name="Kt_sb")
        wkv = Wk_d.rearrange("(a p) c -> p a c", p=128)
        ktv = Kt_d.rearrange("(a p) c -> p a c", p=128)
        for a in range(8):
            nc.gpsimd.dma_start(Wk_sb[:, a, 0:512], wkv[:, a, 0:512])
            nc.sync.dma_start(Wq_sb[:, a, 512:1024], wqv[:, a, 512:1024])
            nc.sync.dma_start(Kt_sb[:, a], ktv[:, a])
            nc.sync.dma_start(Wk_sb[:, a, 512:1024], wkv[:, a, 512:1024])

        qT_sb = sb.tile([128, 4, L], BF16, name="qT_sb")
        kT_sb = sb.tile([128, 4, L], BF16, name="kT_sb")
        ctxT_sb = sb.tile([128, 4, L], BF16, name="ctxT_sb")
        sigq_sb = sb.tile([128, 8, SIGW], BF16, name="sigq_sb")
        sigk_sb = sb.tile([128, 8, SIGW], BF16, name="sigk_sb")
        if use_tres:
            from concourse.masks import make_identity
            ident_sb = sb.tile([128, 128], BF16, name="ident_sb")
            make_identity(nc, ident_sb)
        vpa_sb = sb.tile([128, 8, 520], BF16, name="vpa_sb")
        vpa_r = vpa_sb.rearrange("p lb (hh c) -> p lb hh c", c=65)
        nc.vector.memset(vpa_r[:, :, :, 64], 1.0)  # the denominator "ones" column

        # staging DRAM in scramble layout [jh, il, ihi, jl]
        qp2_d = dr.tile([16, 64, 16, 64], BF16, name="qp2_d")
        kp2_d = dr.tile([16, 64, 16, 64], BF16, name="kp2_d")

        # ---- phase helpers ---------------------------------------------
        def proj_tile(Xt_sb, W_sb, bn_sb, x2_d, pfx, wq, cb, lb):
            """One l/c tile of x = X @ W (+b), staged to DRAM in scramble layout."""
            x2w = x2_d.rearrange("jh il ihi jl -> il jh (ihi jl)")
            ps = psA.tile([128, 512], F32, tag="mm", bufs=2,
                          name=f"ps_{pfx}_{cb}_{lb}")
            if not zero_bias:
                nc.tensor.matmul(ps, ones1_sb[0:1, :],
                                 bn_sb[0:1, cb * 512:cb * 512 + 512],
                                 start=True, stop=False)
            for dm in range(8):
                nc.tensor.matmul(
                    ps,
                    Xt_sb[:, dm, lb * 128:lb * 128 + 128],
                    W_sb[:, dm, cb * 512:cb * 512 + 512],
                    start=(zero_bias and dm == 0), stop=(dm == 7))
            st = sb.tile([128, 512], BF16, tag="stage", bufs=10,
                         name=f"st_{pfx}_{cb}_{lb}")
            nc.vector.tensor_copy(st, ps)
            # st partition p = 16*il2 + jh iterates in the same linear
            # order as the dst view's (il2, jh) dims
            wq.dma_start(
                x2w[8 * lb:8 * lb + 8, :, 512 * cb:512 * cb + 512], st)

        def proj_nat(Xt_sb, W_sb, bn_sb, x2_d, pfx, wq):
            for cb in range(2):
                for lb in range(8):
                    proj_tile(Xt_sb, W_sb, bn_sb, x2_d, pfx, wq, cb, lb)

        def scramble(x2_d, sig_sb, pfx, pool_share, dts=range(8)):
            """sig[64 hib + il, dt, 64 + 64 jh + jl] <- qp[16 il + jh, 64 ihi + jl]
            (ihi = 2 dt + hib); boundary cols 62:64 fixed up separately."""
            x2g = x2_d.rearrange("jh il ihi jl -> il jh ihi jl")
            for dt in dts:
                for hib in range(2):
                    ihi = 2 * dt + hib
                    dst = sig_sb[64 * hib:64 * hib + 64, dt, 64:64 + 64 * njh]
                    q = nc.gpsimd if (pool_share and dt < 4) else nc.scalar
                    q.dma_start(
                        dst.rearrange("p (jh jl) -> p jh jl", jl=64),
                        x2g[:, 0:njh, ihi])
                    if not use_tres:
                        # flen=4: cols 62:64 (j = 1022:1024) lie outside the
                        # gathered bands; fetch the 2-col wrap strip directly.
                        nc.scalar.dma_start(sig_sb[64 * hib:64 * hib + 64, dt, 62:64],
                                            x2g[:, 15, ihi, 62:64])

        def sig_fixup(sig_sb):
            if use_tres:
                # wrap cols 62:64 duplicate cols 1086:1088; apply pad mask em
                nc.gpsimd.tensor_tensor(
                    sig_sb[:, :, 62:64], sig_sb[:, :, 1086:1088],
                    em_sb[:, None, 0:2].to_broadcast((128, 8, 2)), OP.mult)
            else:
                nc.gpsimd.tensor_tensor(
                    sig_sb[:, :, 62:64], sig_sb[:, :, 62:64],
                    em_sb[:, None, 0:2].to_broadcast((128, 8, 2)), OP.mult)
                nc.gpsimd.tensor_tensor(
                    sig_sb[:, :, 576:578], sig_sb[:, :, 576:578],
                    em_sb[:, None, 2:4].to_broadcast((128, 8, 2)), OP.mult)

        def proj_T(W_sb, Xt_sb, bt_sb, T_sb, pfx):
            """xT = (X @ W + b).T for program channels [0,512) -> head tiles."""
            for ct in range(4):
                for q2 in range(2):
                    ps = psA.tile([128, 512], F32, tag="mm", bufs=2,
                                  name=f"psT_{pfx}_{ct}_{q2}")
                    for dm in range(8):
                        nc.tensor.matmul(
                            ps,
                            W_sb[:, dm, ct * 128:ct * 128 + 128],
                            Xt_sb[:, dm, q2 * 512:q2 * 512 + 512],
                            start=(dm == 0), stop=(dm == 7))
                    nc.scalar.activation(
                        T_sb[:, ct, q2 * 512:q2 * 512 + 512], ps,
                        AF.Identity, bias=bt_sb[:, ct:ct + 1], scale=1.0)

        def conv_quarter(sig_sb, cw_d, T_sb, pfx, ldq, quarter, cpq=None):
            """One o'-quarter of: T[c,l] += conv output, scrambled back into
            head-transposed tiles.
            conv psum tile s: partition o' = 128 s + 64 ph + k, free l2' = 64 hl + m."""
            Tr = T_sb.rearrange("p q (m r) -> p q m r", r=16)
            if True:
                pss = [psA.tile([128, 512], F32, tag="conv", bufs=3,
                                name=f"cps_{pfx}_{quarter}_{si}") for si in range(2)]
                tres = None

                def emit_tres(si):
                    # residual: tres[o', l2'] = sig[d=l2', t=o'] via PE transpose
                    sblk = 2 * quarter + si
                    for lb2 in range(4):
                        nc.tensor.matmul(
                            tres[si][:, 128 * lb2:128 * lb2 + 128],
                            sig_sb[:, lb2, 64 + 128 * sblk:64 + 128 * sblk + 128],
                            ident_sb, is_transpose=True,
                            start=(lb2 == 0), stop=(lb2 == 3))

                if use_tres:
                    tres = [psA.tile([128, 512], BF16, tag="tres", bufs=3,
                                     name=f"tres_{pfx}_{quarter}_{si}") for si in range(2)]
                    emit_tres(0)

                cwts = []

                def cps_mm(si, dt):
                    for f in range(flen):
                        nc.tensor.matmul(
                            pss[si],
                            cwts[dt >> 1][:, dt & 1,
                                          f * 256 + si * 128:f * 256 + si * 128 + 128],
                            sig_sb[:, dt, 62 + f:62 + f + 512],
                            start=(dt == 0 and f == 0),
                            stop=(dt == 7 and f == flen - 1))

                def emit_adds(si):
                    sblk = 2 * quarter + si
                    ps = pss[si].rearrange("p (q h m) -> p q h m", q=4, h=2)
                    for ph in range(2):
                        for pe in range(2):
                            dst = Tr[64 * pe:64 * pe + 64, :, :, 2 * sblk + ph]
                            if use_tres:
                                # hw: ops may read only ONE psum operand, and
                                # gpsimd cannot touch psum — so: DVE copy of
                                # the transposed residual, then in-place add
                                tr2 = tres[si].rearrange(
                                    "p (q h m) -> p q h m", q=4, h=2)
                                if cpq is None:
                                    nc.vector.tensor_copy(
                                        dst, tr2[64 * ph:64 * ph + 64, :, pe, :])
                                else:
                                    cpq.activation(
                                        dst, tr2[64 * ph:64 * ph + 64, :, pe, :],
                                        AF.Copy)
                                nc.vector.tensor_tensor(
                                    dst, ps[64 * ph:64 * ph + 64, :, pe, :], dst,
                                    OP.add)
                            else:
                                nc.vector.tensor_tensor(
                                    dst, ps[64 * ph:64 * ph + 64, :, pe, :], dst,
                                    OP.add)

                # cw weights arrive as dt-pair DMAs (partition-remapped from
                # the (dt, p, c) DRAM layout) to halve issue/HWDGE overhead
                qsl = slice(quarter * flen * 256, (quarter + 1) * flen * 256)

                def load_pair(d2):
                    cwt = sb.tile([128, 2, flen * 256], BF16, tag="cw", bufs=4,
                                  name=f"cw_{pfx}_{quarter}_{d2}")
                    ldq.dma_start(cwt, cw_d[2 * d2:2 * d2 + 2, :, qsl].rearrange("dt p c -> p dt c"))
                    cwts.append(cwt)

                # si=0 leads by four dt-blocks so si=1's psum-slot wait and
                # tres si=1's are absorbed by real work; si=0 stops (and its
                # adds are emitted) before si=1's tail
                for d2 in range(2):
                    load_pair(d2)
                    cps_mm(0, 2 * d2)
                    cps_mm(0, 2 * d2 + 1)
                for dt in range(4):
                    cps_mm(1, dt)
                if use_tres:
                    emit_tres(1)
                for d2 in range(2, 4):
                    load_pair(d2)
                    cps_mm(0, 2 * d2)
                    cps_mm(0, 2 * d2 + 1)
                emit_adds(0)
                for dt in range(4, 8):
                    cps_mm(1, dt)
                emit_adds(1)


        def vp_tiles(Vt_sb, Wv_sb, lbs):
            for lb in lbs:
                ps = psA.tile([128, 512], F32, tag="mm", bufs=2, name=f"psv_{lb}")
                if not zero_bias:
                    nc.tensor.matmul(ps, ones1_sb[0:1, :], bVn_sb[0:1, :],
                                     start=True, stop=False)
                for dm in range(8):
                    nc.tensor.matmul(ps, Vt_sb[:, dm, lb * 128:lb * 128 + 128],
                                     Wv_sb[:, dm, :],
                                     start=(zero_bias and dm == 0), stop=(dm == 7))
                nc.vector.tensor_copy(
                    vpa_r[:, lb, :, 0:64],
                    ps.rearrange("p (hh c) -> p hh c", hh=8))

        # ---- phase sequence --------------------------------------------
        proj_nat(Qt_sb, Wq_sb, bQn_sb, qp2_d, "q", nc.scalar)
        scramble(qp2_d, sigq_sb, "q", pool_share=True)
        if not use_tres:
            proj_T(Wq_sb, Qt_sb, bQt_sb, qT_sb, "q")

        # V-side loads (SP) before the cw streams; Vt's qkvt WAR is satisfied
        # by the time SP reaches it
        Vt_sb = sb.tile([128, 8, L], BF16, tag="qkvt", bufs=2, name="Vt_sb")
        nc.sync.dma_start(Vt_sb, Vt_d.rearrange("(a p) c -> p a c", p=128))
        Wv_sb = sb.tile([128, 8, 512], BF16, name="Wv_sb")
        nc.sync.dma_start(Wv_sb, Wv_d.rearrange("(a p) c -> p a c", p=128))
        Wfc_sb = sb.tile([128, 4, DM], BF16, name="Wfc_sb")
        nc.sync.dma_start(Wfc_sb, Wfc_d.rearrange("(t p) c -> p t c", p=128))

        sig_fixup(sigq_sb)
        # manual interleave: 4 proj_k tiles per conv_q quarter so the DVE
        # queue (st copies | conv adds) drains in execution order
        ktiles = [(cb, lb) for cb in range(2) for lb in range(8)]
        for seg in range(4):
            for cb, lb in ktiles[4 * seg:4 * seg + 4]:
                proj_tile(Kt_sb, Wk_sb, bKn_sb, kp2_d, "k", nc.scalar, cb, lb)
            if seg == 1:
                scramble(kp2_d, sigk_sb, "k", pool_share=False, dts=range(4))
            if seg == 3:
                scramble(kp2_d, sigk_sb, "k", pool_share=False, dts=range(4, 8))
            conv_quarter(sigq_sb, cwq_d, qT_sb, "q", nc.sync, seg)
        if not use_tres:
            proj_T(Wk_sb, Kt_sb, bKt_sb, kT_sb, "k")
        sig_fixup(sigk_sb)
        # vp tiles interleave with conv_k quarters (DVE: vpa copies | adds)
        for seg in range(4):
            vp_tiles(Vt_sb, Wv_sb, range(2 * seg, 2 * seg + 2))
            conv_quarter(sigk_sb, cwk_d, kT_sb, "k", nc.sync, seg)

        psA.release()
        psB = tc.alloc_tile_pool(name="psB", bufs=1, space="PSUM")

        # ---- attention + fc, interleaved by query half ------------------
        odv = out_d.rearrange("(lb p) c -> p lb c", p=128)

        def score_tiles(qb, p4, rng):
            """Emit score matmuls + exp for (kt2, pe) pairs in rng; fill pts."""
            pt_tiles = _pts[(qb, p4)]
            for idx in rng:
                kt2, pe = idx >> 1, idx & 1
                ps_st = psB.tile([128, 1024], F32, tag="st", bufs=2,
                                 name=f"st_{p4}_{qb}_{kt2}_{pe}")
                for h in range(2):
                    kt = 2 * kt2 + h
                    nc.tensor.matmul(
                        ps_st[:, 512 * h:512 * h + 512],
                        kT_sb[64 * pe:64 * pe + 64, p4, kt * 128:kt * 128 + 128],
                        qT_sb[64 * pe:64 * pe + 64, p4, qb * 512:qb * 512 + 512],
                        start=True, stop=True, tile_position=(64 * pe, 0))
                pt = sb.tile([128, 1024], BF16, tag="pt", bufs=9,
                             name=f"pt_{p4}_{qb}_{kt2}_{pe}")
                nc.scalar.activation(pt, ps_st, AF.Exp, scale=0.125)
                pt_tiles[pe][2 * kt2] = pt[:, 0:512]
                pt_tiles[pe][2 * kt2 + 1] = pt[:, 512:1024]

        def ctx_half(qb, p4, pe):
            pt_tiles = _pts[(qb, p4)]
            hl = 2 * p4 + pe
            ps_ctx = psB.tile([65, 512], F32, tag="mm2", bufs=3,
                              name=f"ctx_{p4}_{qb}_{pe}")
            for kt in range(8):
                nc.tensor.matmul(
                    ps_ctx, vpa_sb[:, kt, 65 * hl:65 * hl + 65],
                    pt_tiles[pe][kt], start=(kt == 0), stop=(kt == 7))
            rc = sb.tile([1, 512], F32, tag="recip", bufs=2,
                         name=f"rc_{p4}_{qb}_{pe}")
            nc.vector.reciprocal(rc, ps_ctx[64:65, :])
            rcb = sb.tile([1, 512], BF16, tag="recipb", bufs=2,
                          name=f"rcb_{p4}_{qb}_{pe}")
            nc.vector.tensor_copy(rcb, rc)
            ps_bc = psB.tile([64, 512], F32, tag="bc", bufs=1,
                             name=f"bc_{p4}_{qb}_{pe}")
            nc.tensor.matmul(ps_bc, ones1_sb[0:1, 0:64], rcb,
                             start=True, stop=True)
            bc_sb = sb.tile([64, 512], BF16, tag="bcs", bufs=2,
                            name=f"bcs_{p4}_{qb}_{pe}")
            nc.vector.tensor_copy(bc_sb, ps_bc)
            nc.vector.tensor_tensor(
                ctxT_sb[64 * pe:64 * pe + 64, p4, qb * 512:qb * 512 + 512],
                ps_ctx[0:64, :], bc_sb, OP.mult)

        def fc_tile(lb, db):
            ps = psB.tile([128, 512], F32, tag="mm2", bufs=3,
                          name=f"fc_{lb}_{db}")
            for t4 in range(4):
                nc.tensor.matmul(
                    ps, ctxT_sb[:, t4, lb * 128:lb * 128 + 128],
                    Wfc_sb[:, t4, db * 512:db * 512 + 512],
                    start=(t4 == 0), stop=(t4 == 3))
            st = sb.tile([128, 512], BF16, tag="ostage", bufs=4,
                         name=f"ost_{lb}_{db}")
            nc.vector.tensor_copy(st, ps)
            nc.sync.dma_start(odv[:, lb, db * 512:db * 512 + 512], st)

        # software-pipelined attention: group g's ctx/bc/fc interleaves with
        # group g+1's score matmuls so the Act exp stream never starves
        groups = [(0, p4) for p4 in range(4)] + [(1, p4) for p4 in range(4)]
        _pts = {g: [[None] * 8 for _ in range(2)] for g in groups}
        # fc tiles assigned to groups: qb1 group p4 -> lb=p4; tail lbs 4-7
        fc_jobs = {g: [] for g in groups}
        for p4 in range(4):
            fc_jobs[(1, p4)] = [(p4, 0), (p4, 1)]
        score_tiles(*groups[0], range(8))
        for i, g in enumerate(groups):
            nxt = groups[i + 1] if i + 1 < len(groups) else None
            ctx_half(*g, 0)
            if nxt:
                score_tiles(*nxt, range(0, 4))
            ctx_half(*g, 1)
            if nxt:
                score_tiles(*nxt, range(4, 8))
            for lb, db in fc_jobs[g]:
                fc_tile(lb, db)
        for lb in range(4, 8):
            for db in range(2):
                fc_tile(lb, db)

        psB.release()
        sb.release()
        dr.release()

    nc.finalize()
    return nc


# ----------------------------------------------------------------------------
# host-side data prep
# ----------------------------------------------------------------------------
def _host_prep(inp, flen):
    """Build the 8 per-core input dicts (core ci = 2*b + g)."""
    # per-parity shared tensors (g = 0, 1)
    shared = []
    for g in range(2):
        pi = np.arange(DM) ^ (512 * g)
        d = {}
        d["WQ"] = np.ascontiguousarray(inp["WQ"][:, pi]).astype(bf16)
        d["WK"] = np.ascontiguousarray(inp["WK"][:, pi]).astype(bf16)
        d["WV"] = np.ascontiguousarray(inp["WV"][:, 512 * g:512 * g + 512]).astype(bf16)
        d["Wfc"] = np.ascontiguousarray(inp["Wfc"][512 * g:512 * g + 512, :]).astype(bf16)
        bQ = inp["bQ"][pi].astype(np.float32)
        bK = inp["bK"][pi].astype(np.float32)
        bV = inp["bV"][512 * g:512 * g + 512].astype(np.float32)
        d["bQn"] = bQ[None, :].astype(bf16)
        d["bKn"] = bK[None, :].astype(bf16)
        d["bVn"] = bV[None, :].astype(bf16)
        d["bQt"] = np.ascontiguousarray(bQ[:512].reshape(4, 128).T).astype(np.float32)
        d["bKt"] = np.ascontiguousarray(bK[:512].reshape(4, 128).T).astype(np.float32)
        for name, key in (("cwq", "conv_q"), ("cwk", "conv_k")):
            c = np.asarray(inp[key])[:, :, :flen].astype(np.float32)  # (d, o, f)
            c = np.ascontiguousarray(c.transpose(2, 0, 1))            # (f, d, o)
            c = c[:, pi, :][:, :, pi]
            # layout (8 dt, 128 p, 4 quarter, flen f, 256): column grouping so
            # each conv pass loads only its own o'-quarter of the weights
            c = c.transpose(1, 0, 2).reshape(8, 128, flen, 4, 256)
            c = np.ascontiguousarray(c.transpose(0, 1, 3, 2, 4)).reshape(8, 128, flen * 1024)
            d[name] = c.astype(bf16)
        em = np.zeros((128, 4), np.float32)
        em[:, :] = np.array([0, 0, 1, 1], np.float32) if g == 0 else \
            np.array([1, 1, 0, 0], np.float32)
        d["emask"] = em
        shared.append(d)

    maps = []
    for b in range(B):
        for g in range(2):
            sigma = np.arange(L) ^ (8 * g)
            m = dict(shared[g])
            m["Qt"] = np.ascontiguousarray(np.asarray(inp["Q"])[b][sigma, :].T).astype(bf16)
            m["Kt"] = np.ascontiguousarray(np.asarray(inp["K"])[b][sigma, :].T).astype(bf16)
            m["Vt"] = np.ascontiguousarray(np.asarray(inp["V"])[b][sigma, :].T).astype(bf16)
            maps.append(m)
    return maps


def _combine(results, inp):
    out = np.zeros((B, L, DM), np.float32)
    for b in range(B):
        for g in range(2):
            sigma = np.arange(L) ^ (8 * g)
            out[b] += np.asarray(results[2 * b + g]["out"])[sigma, :].astype(np.float32)
        out[b] += np.asarray(inp["bfc"], dtype=np.float32)
    return out


def _get_program(flen, zero_bias=False):
    key = (flen, zero_bias)
    if key not in _CACHE:
        _CACHE[key] = _build(flen, zero_bias=zero_bias)
    return _CACHE[key]


def run_on_cores(inputs, trace=False):
    """Run the SPMD kernel; returns (full_output, BassKernelResults)."""
    from concourse.bass_utils import run_bass_kernel_spmd
    inp = {k: np.asarray(v) for k, v in inputs.items()}
    f_s = np.array(FILTER_LENGTHS, np.float32)
    flen = int(FILTER_LENGTHS[int(np.argmax(f_s * np.asarray(inp["w"], np.float32)))])
    zb = all(not np.any(np.asarray(inp[k])) for k in ("bQ", "bK", "bV"))
    nc = _get_program(flen, zero_bias=zb)
    in_maps = _host_prep(inp, flen)
    res = run_bass_kernel_spmd(nc, in_maps, list(range(N_CORES)), trace=trace)
    return _combine(res.results, inp), res


def kernel(**inputs) -> np.ndarray:
    out, _ = run_on_cores(inputs, trace=False)
    return out
